# revision 47
# baseline (speedup 1.0000x reference)
"""DBSCAN neighbor-count kernel for Trainium2 (8 NeuronCores) — v5 "banded".

Problem: point_features [4, 8192, 16] f32 -> labels [4, 8192] int32
  d2[b,i,j] = ||x_i - x_j||^2 ; neighbor iff dist < 0.5 (d2 < 0.25)
  label = -1 if neighbor_count < 10 else 0

Strategy (sorted band + triangle symmetry):
  - Per batch, sort points by key = x[:, 0] (host). For any pair,
    |key_i - key_j| <= dist, so all near pairs (dist < 0.5) satisfy
    |dkey| < 0.5. In sorted order the neighbor candidates of row-block t
    (rows [128t, 128t+128)) lie in a contiguous window [128t, hi_t) with
    hi_t = first j with key_j >= key_{last row of t} + 0.5. Only the
    upper-triangle band is computed (~14M pairs/batch vs 67M dense):
      * rowsum over the window counts self + all j >= block start
      * near pairs (i, j) with i in an EARLIER block are recovered from
        column sums of the window masks (d2 symmetric), excluding the
        128 diagonal columns (those pairs are already fully counted by
        the in-block rowsums since the diag block computes both orders).
  - 8 cores: core c -> batch b=c//2, parity r=c%2; core takes global
    blocks t = 2*li + r, li = 0..31 (interleaved for load balance).
    SPMD runs ONE program on all cores, so the window profile is the
    per-li MAX over all 8 cores, and all column indexing is in
    parity-relative coordinates (rel = global - 128r); the host shifts
    each core's rhs points by 128r so the instruction stream is
    core-invariant.
  - Threshold folded into an augmented Gram matmul (K=18):
      Gt[i,j] = dot(x_i,x_j) - a_i - b_j > 0  <=>  neighbor
    Gram matmuls use 4x PE row tiling (strip = li % 4, K=18 <= 32).
  - Epilogue per [128, <=1024] PSUM group (3-deep ring), split across
    both PSUM engines by a deterministic greedy balance (ScalarE has a
    large measured per-op overhead, DVE almost none):
      ScalarE: activation(Sigmoid, scale=1e6, accum_out) -> row-count
      VectorE: tensor_scalar(is_gt 0, accum_out)         -> row-count
    The mandatory elementwise output IS the 0/1 mask (fp8, SBUF).
    TensorE column-sums mask pieces (rows below the diagonal block are
    simply never referenced) into per-512-chunk PSUM accumulators with
    M=1 ones-stationary matmuls (1-column weight loads keep the PE
    pipeline hot); chunk bursts are emitted at block boundaries with a
    2-block delay, and V/S copies each finished chunk row to SBUF, one
    colsum DMA at the end. parts accumulators are split per engine to
    avoid cross-engine WAW tracking.
  - Host merges: counts = own-block rowsums + both cores' column sums
    (coverage-masked); label = -1 iff count < 9.5. Sorted -> original
    order via the argsort permutation.

The program structure depends on the window profile (data dependent);
compilation is cached per (window profile, repeat). All DRAM I/O shapes
are static.
"""
import numpy as np
import ml_dtypes

import jax
from jax.experimental.shard_map import shard_map
from jax.sharding import Mesh, PartitionSpec

import bass_rust
import concourse.bass as bass
import concourse.mybir as mybir
import concourse.tile as tile
from concourse.bass2jax import (
    _bass_exec_p,
    fast_dispatch_compile,
    install_neuronx_cc_hook,
    partition_id_tensor,
)

B, N, D = 4, 8192, 16
KAUG = 18                  # features + threshold-fold rows
NBLK = 32                  # local row blocks per core (128 rows each)
CHUNK = 512                # column chunk / colsum accumulator width
GCAP = 1024                # PSUM tail group cap (2 banks)
NRQ = N + CHUNK            # rhs width incl kill padding
NPARTS = 128               # rowsum slots
N_CORES = 8
DELAY = 2                  # blocks between mask production and colsum burst
MASK_DT = mybir.dt.float8e4
MASK_NP = ml_dtypes.float8_e4m3
KILL = -1.0e9

_cache = {}

# Experiment knobs (cache key includes these). eng_mode: greedy|scalar|vector
CFG = dict(ones_m1=True, eng_mode="greedy", skip_colsum=False,
           skip_epilogue=False, dve_factor=1.5,
           no_copy=False, delay=2, gcap=1024, tail_bufs=3, copy_eng="greedy",
           parts_split=True, bursts_per_block=2, acc_bufs=2, block_eng=False,
           fold_waits=False, ring_mode="single", cs_mode="row")

_FOLD_OK = {
    "InstMatmult", "InstLdweights", "InstActivation", "InstTensorScalarPtr",
    "InstTensorReduce", "InstTensorCopy", "InstMemset", "InstTensorTensor",
}


def split_excess_waits(nc, limit=1, fold=True):
    """This walrus build caps sync-waits per instruction. Move extras onto the
    immediately-preceding same-engine instruction when it carries no wait
    (earlier wait = semantically stronger, same stall point); otherwise hoist
    into standalone NoOps on the same engine."""
    n_split = n_fold = 0
    for f in nc.m.functions:
        for b in f.blocks:
            out = []
            changed = False
            last_by_eng = {}
            for i in b.instructions:
                si = i.sync_info
                if si and si.on_wait and len(si.on_wait) > limit:
                    waits = list(si.on_wait)
                    extra, keep = waits[:-limit], waits[-limit:]
                    rest = []
                    for w in extra:
                        prev = last_by_eng.get(i.engine)
                        psi = prev.sync_info if prev is not None else None
                        if (fold and prev is not None
                                and type(prev).__name__ in _FOLD_OK
                                and (psi is None or not psi.on_wait)):
                            upd = list(psi.on_update) if psi and psi.on_update else []
                            prev.sync_info = bass_rust.SyncInfo(
                                on_wait=[w], on_update=upd)
                            n_fold += 1
                        else:
                            rest.append(w)
                    for k, w in enumerate(rest):
                        nop = mybir.InstNoOp(name=f"{i.name}_xw{k}")
                        nop.engine = i.engine
                        nop.sync_info = bass_rust.SyncInfo(on_wait=[w], on_update=[])
                        out.append(nop)
                        last_by_eng[i.engine] = nop
                        n_split += 1
                    si.on_wait = keep
                    i.sync_info = si
                    changed = True
                out.append(i)
                last_by_eng[i.engine] = i
            if changed:
                b.instructions = out
    return n_split, n_fold


# ---------------------------------------------------------------------------
# Geometry: shared (core-invariant) band profile from the input data.
# ---------------------------------------------------------------------------
def _geometry(x):
    """x: [B, N, D] f32. Returns (orders, profile) where profile is a
    hashable tuple (W, hi) of per-local-block window widths / ends in
    parity-relative columns, maxed over all 8 cores."""
    orders = []
    W = np.zeros(NBLK, np.int64)
    hi = np.zeros(NBLK, np.int64)
    for b in range(B):
        o = np.argsort(x[b, :, 0], kind="stable")
        orders.append(o)
        k = np.ascontiguousarray(x[b, o, 0])
        for li in range(NBLK):
            for r in range(2):
                t = 2 * li + r
                end = int(np.searchsorted(
                    k, k[128 * t + 127] + np.float32(0.5), side="left"))
                rel_end = end - 128 * r
                w = rel_end - 256 * li
                w = max(w, 1)
                w = -(-w // CHUNK) * CHUNK
                W[li] = max(W[li], w)
                hi[li] = max(hi[li], rel_end)
    hi = np.minimum(hi, 256 * np.arange(NBLK) + W)
    # diag must at least be covered
    assert (W >= 512).all() and (W % CHUNK == 0).all()
    return orders, (tuple(int(v) for v in W), tuple(int(v) for v in hi))


def _plan(profile):
    """Static per-program plan from the window profile."""
    W, hi = profile
    # colsum pieces per relative 512-chunk: (li, rel_a, rel_b)
    nchunk = NRQ // CHUNK
    pieces = [[] for _ in range(nchunk)]
    for li in range(NBLK):
        a0, b0 = 256 * li + 128, int(hi[li])   # diag cols excluded
        c = a0 // CHUNK
        while c * CHUNK < b0:
            a = max(a0, c * CHUNK)
            b = min(b0, (c + 1) * CHUNK)
            if a < b:
                pieces[c].append((li, a, b))
            c += 1
    last_li = [max((p[0] for p in pieces[c]), default=-1) for c in range(nchunk)]
    # coverage of colsum outputs (host zeroes the rest)
    cov = np.zeros(NRQ, bool)
    for c in range(nchunk):
        for (li, a, b) in pieces[c]:
            cov[a:b] = True
    # per-block epilogue piece plan: (off, plen, eng). S consumes its own
    # [128,1024] PSUM ring, D its own [128,512] ring; greedy balances
    # projected finish times. Deterministic, so merge can re-derive slots.
    eng_t = [0.0, 0.0]
    groups = []
    for li in range(NBLK):
        fdeb = min(int(W[li]), -(-(int(hi[li]) - 256 * li) // 128) * 128)
        g, off = [], 0
        while off < fdeb:
            rem = fdeb - off
            if CFG["eng_mode"] == "scalar":
                eng = "A"
            elif CFG["eng_mode"] == "vector":
                eng = "D"
            else:
                sf = eng_t[0] + _s_cost(min(1024, rem))
                df = eng_t[1] + _d_cost(min(512, rem)) * CFG["dve_factor"]
                eng = "A" if sf <= df else "D"
            dcap = 1024 if CFG["ring_mode"] == "single" else 512
            plen = min(1024 if eng == "A" else dcap, rem)
            if eng == "A":
                eng_t[0] += _s_cost(plen)
            else:
                eng_t[1] += _d_cost(plen) * CFG["dve_factor"]
            g.append((off, plen, eng))
            off += plen
        groups.append(g)
    nslot = sum(len(g) for g in groups)
    assert nslot <= NPARTS, nslot
    # colsum chunks are processed in pairs (2a, 2a+1) -> one [1,1024] acc
    npair = (nchunk + 1) // 2
    pair_last = [max(last_li[2 * a], last_li[min(2 * a + 1, nchunk - 1)])
                 for a in range(npair)]
    # mask pool sizing: mask li is last read at emission block
    emit_at = [min(pair_last[c // 2] + CFG["delay"], NBLK - 1)
               for c in range(nchunk)]
    death = [0] * NBLK
    for c in range(nchunk):
        for (li, a, b) in pieces[c]:
            death[li] = max(death[li], emit_at[c])
    live_max = 0
    for li in range(NBLK):
        live = sum(1 for j in range(li + 1) if death[j] >= li)
        live_max = max(live_max, live)
    mask_bufs = live_max + 2
    wmask = int(max(W))
    return dict(W=W, hi=hi, pieces=pieces, last_li=last_li, cov=cov,
                groups=groups, nslot=nslot, mask_bufs=mask_bufs, wmask=wmask,
                nchunk=nchunk, npair=npair, pair_last=pair_last)


# ---------------------------------------------------------------------------
# Builder
# ---------------------------------------------------------------------------
# epilogue/copy cost models (ns) for greedy engine balancing; constants
# fitted from allS/allD ablations on this kernel (ScalarE has a large
# per-op overhead here; DVE shows nearly none).
def _s_cost(fd):
    return (450 + fd) / 1.2


def _d_cost(fd):
    return (5 + fd) / 0.96


def _build_v5(profile, repeat=1):
    plan = _plan(profile)
    W, hi = plan["W"], plan["hi"]
    groups, pieces = plan["groups"], plan["pieces"]
    last_li = plan["last_li"]
    wmask, mask_bufs = plan["wmask"], plan["mask_bufs"]
    nchunk, npair, pair_last = plan["nchunk"], plan["npair"], plan["pair_last"]

    bf16 = mybir.dt.bfloat16
    f32 = mybir.dt.float32
    SIG = mybir.ActivationFunctionType.Sigmoid
    CPY = mybir.ActivationFunctionType.Copy

    nc = bass.Bass()
    ls_d = nc.dram_tensor("lhsT", [KAUG, NBLK * 128], bf16, kind="ExternalInput")
    rq_d = nc.dram_tensor("rhs", [KAUG, NRQ], bf16, kind="ExternalInput")
    parts_d = nc.dram_tensor("parts", [128, NPARTS], f32, kind="ExternalOutput")
    parts2_d = nc.dram_tensor("parts2", [128, NPARTS], f32,
                              kind="ExternalOutput")
    colsum_d = nc.dram_tensor("colsum", [1, NRQ], f32, kind="ExternalOutput")

    with tile.TileContext(nc) as tc:
        with (
            tc.tile_pool(name="inp", bufs=1) as inp,
            tc.tile_pool(name="masks", bufs=mask_bufs) as maskp,
            tc.tile_pool(name="cs", bufs=3) as csp,
            tc.tile_pool(name="fin", bufs=1) as fin,
            tc.tile_pool(name="tailS",
                         bufs=(3 if CFG["ring_mode"] == "single" else 2),
                         space="PSUM") as tailSp,
            tc.tile_pool(name="tailD", bufs=2,
                         space="PSUM") as tailDp,
            tc.tile_pool(name="acc", bufs=CFG["acc_bufs"], space="PSUM") as accp,
        ):
            # double-buffered inputs (explicit, so padding rows are zeroed
            # once). lhsT is replicated to all 4 PE strips so every 512-col
            # matmul can round-robin strips (4x tile concurrency).
            ls_sb = [inp.tile([128, NBLK * 128], bf16, name=f"ls{i}")
                     for i in range(2)]
            rq_sb = [inp.tile([128, NRQ], bf16, name=f"rq{i}") for i in range(2)]
            ones = fin.tile([128, 128], MASK_DT, name="ones")
            colsum_sb = fin.tile([1, NRQ], f32, name="colsum")
            parts = fin.tile([128, NPARTS], f32, name="parts")
            parts2 = fin.tile([128, NPARTS], f32, name="parts2")

            # zero K-padding rows (18..31 of each strip) once; per-rep DMAs
            # only touch rows 0..17. Spread across engines.
            for i in range(2):
                nc.gpsimd.memset(ls_sb[i][:, :].bitcast(mybir.dt.uint32), 0)
                nc.scalar.memzero(rq_sb[i][:, :])
            nc.gpsimd.memset(ones, 1.0)

            nc.gpsimd.memset(parts[:, :].bitcast(mybir.dt.uint32), 0)
            nc.gpsimd.memset(parts2[:, :].bitcast(mybir.dt.uint32), 0)
            nc.gpsimd.memset(colsum_sb[:, :].bitcast(mybir.dt.uint32), 0)

            def epilogue(pt, sl, mask, force=None):
                fd = pt.shape[-1]
                eng = force if force is not None else pick(fd)
                if eng == "A":
                    nc.scalar.activation(mask, pt, SIG, bias=0.0,
                                         scale=1.0e6,
                                         accum_out=parts[:, sl:sl + 1])
                else:
                    p2 = parts2 if CFG["parts_split"] else parts
                    nc.vector.tensor_scalar(mask, pt, 0.0, None,
                                            mybir.AluOpType.is_gt,
                                            mybir.AluOpType.add,
                                            accum_out=p2[:, sl:sl + 1])

            cp_alt = [0]

            def copy_row(dst, src):
                ce = CFG["copy_eng"]
                if ce == "S":
                    eng = "A"
                elif ce == "D":
                    eng = "D"
                else:
                    eng = "A" if cp_alt[0] % 2 == 0 else "D"
                    cp_alt[0] += 1
                if eng == "A":
                    nc.scalar.activation(dst, src, CPY)
                else:
                    nc.vector.tensor_copy(dst, src)

            def body(rep):
                ls, rq = ls_sb[rep % 2], rq_sb[rep % 2]
                sfx = f"_r{rep}"
                for s in range(4):
                    nc.sync.dma_start(out=ls[32 * s:32 * s + KAUG, :],
                                      in_=ls_d[:, :])
                    nc.sync.dma_start(out=rq[32 * s:32 * s + KAUG, :],
                                      in_=rq_d[:, :])

                masks = {}
                slot = 0
                strip = [0]   # global round-robin strip counter

                ready = []       # chunks whose masks are all produced
                copy_pend = []   # deferred copies
                accg = {}        # group -> (tile, [chunks done])
                live = [c for c in range(nchunk) if pieces[c]]
                gsz = 4 if CFG["cs_mode"] == "bank" else 1
                grp_of = {c: i // gsz for i, c in enumerate(live)}
                grp_members = {}
                for c in live:
                    grp_members.setdefault(grp_of[c], []).append(c)

                def burst(c):
                    # all colsum pieces of one 512-chunk, atomically. In
                    # bank mode four chunks share one PSUM bank at output
                    # partitions 0/32/64/96 (PE column tiling) so ONE
                    # [128,512] V/S copy + one strided-partition DMA
                    # retires four chunks.
                    g = grp_of[c]
                    if g not in accg:
                        shape = ([128, CHUNK] if CFG["cs_mode"] == "bank"
                                 else [1, CHUNK])
                        accg[g] = (accp.tile(shape, f32, tag="acc",
                                             name=f"acc{g}{sfx}"), [])
                    acc, done = accg[g]
                    j = 32 * grp_members[g].index(c)
                    if CFG["cs_mode"] != "bank":
                        j = 0
                    ps = pieces[c]
                    for idx, (li2, a, b) in enumerate(ps):
                        kw = (dict(tile_position=(0, j))
                              if CFG["cs_mode"] == "bank" else {})
                        nc.tensor.matmul(
                            out=acc[j:j + 1, a - c * CHUNK:b - c * CHUNK],
                            lhsT=ones[:, 0:1],
                            rhs=masks[li2][:, a - 256 * li2:b - 256 * li2],
                            start=(idx == 0), stop=(idx == len(ps) - 1),
                            skip_group_check=True, **kw)
                    done.append(c)
                    if len(done) == len(grp_members[g]):
                        copy_pend.append((acc, list(done)))

                def flush_copies():
                    while copy_pend:
                        acc, cs2 = copy_pend.pop(0)
                        if CFG["no_copy"]:
                            continue
                        if CFG["cs_mode"] == "bank":
                            cs = csp.tile([128, CHUNK], f32, tag="cs",
                                          name=f"cs{cs2[0]}{sfx}")
                            copy_row(cs, acc)
                            assert cs2 == list(range(cs2[0], cs2[-1] + 1))
                            v = cs[:, :].rearrange("(a b) f -> a b f", b=32)
                            dv = colsum_d[0:1, :].rearrange(
                                "o (c f) -> (o c) f", f=CHUNK)
                            nc.sync.dma_start(
                                out=dv[cs2[0]:cs2[-1] + 1, :],
                                in_=v[0:len(cs2), 0, :])
                        else:
                            c = cs2[0]
                            copy_row(colsum_sb[0:1, c * CHUNK:
                                               (c + 1) * CHUNK],
                                     acc[0:1, :])

                for li in range(NBLK):
                    mask = maskp.tile([128, wmask], MASK_DT, tag="mask",
                                      name=f"mk{li}{sfx}")
                    masks[li] = mask
                    for (off, plen, eng) in groups[li]:
                        single = CFG["ring_mode"] == "single"
                        pool = tailSp if (single or eng == "A") else tailDp
                        cap = 1024 if (single or eng == "A") else 512
                        tg = "tail" if CFG["ring_mode"] == "single" else "t" + eng
                        pt = pool.tile([128, cap], f32, tag=tg,
                                       name=f"ps{li}_{off}{sfx}")
                        u = 0
                        while u < plen:
                            cl = min(CHUNK, plen - u)
                            s = strip[0] % 4
                            strip[0] += 1
                            c0 = 256 * li + off + u
                            nc.tensor.matmul(
                                out=pt[:, u:u + cl],
                                lhsT=ls[32 * s:32 * s + KAUG,
                                        li * 128:(li + 1) * 128],
                                rhs=rq[32 * s:32 * s + KAUG, c0:c0 + cl],
                                start=True, stop=True,
                                tile_position=(32 * s, 0))
                            u += cl
                        if not CFG["skip_epilogue"]:
                            epilogue(pt[:, :plen], slot,
                                     mask[:, off:off + plen], force=eng)
                        slot += 1
                    # at block boundary: flush deferred copies (their PE
                    # bursts have had a block's worth of time), then emit
                    # at most bursts_per_block chunk bursts
                    flush_copies()
                    if not CFG["skip_colsum"]:
                        for c in range(nchunk):
                            if last_li[c] == li - CFG["delay"] and pieces[c]:
                                ready.append(c)
                        if li == NBLK - 1:
                            for c in range(nchunk):
                                if last_li[c] > li - CFG["delay"] and pieces[c]:
                                    ready.append(c)
                        nb = (len(ready) if li == NBLK - 1
                              else CFG["bursts_per_block"])
                        for _ in range(min(nb, len(ready))):
                            burst(ready.pop(0))
                flush_copies()

                nc.sync.dma_start(out=parts_d[:, :], in_=parts)
                nc.sync.dma_start(out=parts2_d[:, :], in_=parts2)
                if CFG["cs_mode"] == "row":
                    nc.sync.dma_start(out=colsum_d[:, :], in_=colsum_sb)

            for rep in range(repeat):
                body(rep)

    split_excess_waits(nc, fold=CFG["fold_waits"])
    return nc


# ---------------------------------------------------------------------------
# Cached AOT dispatch (compile once, reuse the PJRT executable every call).
# ---------------------------------------------------------------------------
class CompiledBass:
    def __init__(self, nc, n_cores):
        install_neuronx_cc_hook()
        assert nc.dbg_addr is None
        partition_name = (
            nc.partition_id_tensor.name if nc.partition_id_tensor else None)
        in_names, out_names, out_avals = [], [], []
        in_shapes, in_dtypes = [], []
        for alloc in nc.m.functions[0].allocations:
            if not isinstance(alloc, mybir.MemoryLocationSet):
                continue
            name = alloc.memorylocations[0].name
            if alloc.kind == "ExternalInput":
                if name != partition_name:
                    in_names.append(name)
                    in_shapes.append(tuple(alloc.tensor_shape))
                    in_dtypes.append(mybir.dt.np(alloc.dtype))
            elif alloc.kind == "ExternalOutput":
                out_names.append(name)
                out_avals.append(jax.core.ShapedArray(
                    tuple(alloc.tensor_shape), mybir.dt.np(alloc.dtype)))
        self.n_cores = n_cores
        self.in_names = in_names
        self.out_names = out_names
        self.out_shapes = [tuple(a.shape) for a in out_avals]
        self.out_dtypes = [a.dtype for a in out_avals]
        all_in_names = list(in_names)
        if partition_name is not None:
            all_in_names.append(partition_name)

        def _body(*args):
            operands = list(args)
            if partition_name is not None:
                operands.append(partition_id_tensor())
            return tuple(_bass_exec_p.bind(
                *operands, out_avals=tuple(out_avals),
                in_names=tuple(all_in_names),
                out_names=tuple(out_names), lowering_input_output_aliases=(),
                sim_require_finite=True, sim_require_nnan=True, nc=nc))

        devices = jax.devices()[:n_cores]
        assert len(devices) == n_cores, (len(devices), n_cores)
        self.mesh = Mesh(np.asarray(devices), ("core",))
        in_specs = (PartitionSpec("core"),) * len(in_names)
        out_specs = (PartitionSpec("core"),) * len(out_names)
        arg_shapes = [
            jax.ShapeDtypeStruct((n_cores * s[0], *s[1:]), d)
            for s, d in zip(in_shapes, in_dtypes)
        ]

        def compile_fn():
            return jax.jit(
                shard_map(_body, mesh=self.mesh, in_specs=in_specs,
                          out_specs=out_specs, check_rep=False),
                keep_unused=True,
            ).lower(*arg_shapes).compile()

        self.compiled = fast_dispatch_compile(compile_fn)

    def __call__(self, concat_inputs):
        """concat_inputs: np/jax arrays concatenated on axis 0 across cores,
        in self.in_names order. Returns list of per-core output dicts."""
        outs = self.compiled(*concat_inputs)
        res = []
        for c in range(self.n_cores):
            d = {}
            for i, name in enumerate(self.out_names):
                s = self.out_shapes[i]
                d[name] = np.asarray(outs[i]).reshape(self.n_cores, *s)[c]
            res.append(d)
        return res


_state = {}


def get_compiled(repeat=1, profile=None):
    if profile is None:
        profile = _state["profile"]
    key = ("cb5", profile, repeat, tuple(sorted(CFG.items())))
    if key not in _cache:
        _cache[key] = CompiledBass(_build_v5(profile, repeat=repeat), N_CORES)
    return _cache[key]


def _prep_inputs(point_features):
    """Computes geometry, stores it in _state, and returns the per-core
    input blocks concatenated core-major on axis 0."""
    x = np.asarray(point_features, dtype=np.float32)
    orders, profile = _geometry(x)
    plan = _plan(profile)
    _state["profile"] = profile
    _state["orders"] = orders
    _state["plan"] = plan

    xb = x.astype(ml_dtypes.bfloat16)
    xf = xb.astype(np.float32)                      # bf16-rounded features
    sq = np.einsum("bnd,bnd->bn", xf, xf)           # [B, N] f32
    a = sq / 2.0
    nb = (0.25 - sq) / 2.0                          # -b_j

    ls = np.zeros((N_CORES, KAUG, NBLK * 128), np.float32)
    rq = np.zeros((N_CORES, KAUG, NRQ), np.float32)
    for c in range(N_CORES):
        b, r = c // 2, c % 2
        o = orders[b]
        xs = xf[b][o]                               # sorted features
        a_s, nb_s = a[b][o], nb[b][o]
        # lhsT, local-block-major (replicated to all strips on device)
        for li in range(NBLK):
            t = 2 * li + r
            col = li * 128
            rows = slice(128 * t, 128 * t + 128)
            ls[c, 0:D, col:col + 128] = xs[rows].T
            ls[c, D, col:col + 128] = 1.0
            ls[c, D + 1, col:col + 128] = a_s[rows]
        # rhs in parity-relative columns: rel j -> global point j + 128r
        g0 = 128 * r
        nreal = N - g0
        rq[c, 0:D, 0:nreal] = xs[g0:].T
        rq[c, D, 0:nreal] = nb_s[g0:]
        rq[c, D + 1, 0:nreal] = -1.0
        rq[c, D, nreal:] = KILL                     # kill padding
    cast = ml_dtypes.bfloat16
    return {
        "lhsT": ls.reshape(N_CORES * KAUG, NBLK * 128).astype(cast),
        "rhs": rq.reshape(N_CORES * KAUG, NRQ).astype(cast),
    }


def _merge_outputs(res):
    plan = _state["plan"]
    orders = _state["orders"]
    groups, cov = plan["groups"], plan["cov"]
    out = np.empty((B, N), dtype=np.int32)
    for b in range(B):
        counts = np.zeros(N, np.float64)
        for r in range(2):
            core = res[2 * b + r]
            parts = (core["parts"].astype(np.float64)
                     + core["parts2"].astype(np.float64))
            colsum = core["colsum"][0].astype(np.float64)
            slot = 0
            for li in range(NBLK):
                t = 2 * li + r
                rs = np.zeros(128, np.float64)
                for _ in groups[li]:
                    rs += parts[:, slot]
                    slot += 1
                counts[128 * t:128 * t + 128] += rs
            # colsum rel col j -> global sorted col j + 128r
            valid = np.where(cov, colsum, 0.0)
            g_end = min(N, 128 * r + NRQ)
            counts[128 * r:g_end] += valid[:g_end - 128 * r]
        labels_sorted = np.where(counts < 9.5, -1, 0).astype(np.int32)
        lab = np.empty(N, np.int32)
        lab[orders[b]] = labels_sorted
        out[b] = lab
    return out


def kernel(point_features):
    inp = _prep_inputs(point_features)
    cb = get_compiled()
    res = cb([inp[nm] for nm in cb.in_names])
    return _merge_outputs(res)


if __name__ == "__main__":
    x = np.random.default_rng(0).standard_normal((B, N, D)).astype(np.float32)
    y = kernel(x)
    print("out shape/dtype:", y.shape, y.dtype, "uniq:", np.unique(y))
